# revision 46
# baseline (speedup 1.0000x reference)
"""Multi-head attention (B=4, S=2048, E=1024, H=16, hd=64) on 8 TRN2 cores.

Sharding: core c -> batch b = c//2, head-half hh = c%2 (8 heads = 512 internal
dims).  Data parallel on B, tensor parallel on heads.  Each core computes a
partial out-projection for its batch; the host sums the two head-half partials
per batch and adds the (folded) output bias.

Device dataflow (bf16 matmuls, fp32 PSUM accumulation):
  - host pre-transposes q/k/v to (E, S) and casts to bf16 so the projection
    matmuls need no on-chip transpose.  v is staged in eight small
    [128, 8kk, 256seq] tiles that arrive progressively, so the vh projection
    overlaps the k/q staging DMA; khT m-tile 0 and qhT quarter 0 are packed
    into the same TensorE window so attention starts as soon as qT lands.
  - attention per head-PAIR (2g, 2g+1) per 512-query chunk: two K=64
    scoresT matmuls for both heads into one PSUM tile, one Exp over both
    (scale 1/8 pre-folded into Wq/bq host-side), then M=65 AV matmuls
    whose ones-column accumulates the softmax denominator in row 64.
  - division: AV promptly evacuated PSUM->SBUF (frees the accumulator);
    the denominator row's DVE fast-reciprocal is replicated across the 64
    hd partitions by the otherwise-idle GPSIMD engine (partition_broadcast)
    for the deferred steady-state divides; the latency-critical tail slices
    keep a K=1 TensorE broadcast (ones[1,64]^T @ row).  The multiply
    produces attn_outT -- exactly the lhsT needed for the out-projection
    po (q x E) = attn_outT^T @ Wo_loc.
  - ScalarE runs only the exps plus tail evacuations; steady-state
    projection evacuations ride on DVE (tensor_scalar bias-add) since the
    exp stream is the pacing engine.
  - engines run their streams in order, so projection m-tile g+1 matmuls
    are explicitly interleaved into attention pair g's steps to keep
    TensorE busy while ScalarE paces the exps.  Pair 3 has no runnable
    out-proj during its qc0 (out-proj for a query chunk needs all four
    pairs done), so qhT[3] quarters 1-3 are deferred into that slack;
    out-proj(qc) then fills g3's qc+1, with half of out-proj(qc2) riding
    in qc3's steps.
  - a p-state warm-up of 40 DMA-independent dummy matmuls runs during the
    staging window, so the first real chain executes at full TensorE clock.
  - tail (g3 qc3): divide slice 0, out-proj qt10/11 bridging the division
    latency, remaining divides, then the four final out-proj chains.
    Tail chains alternate between sc-tag (score tiles are dead by then)
    and pp-tag PSUM so they never stall on evacuations; final evacs split
    c0->DVE / c1->ScalarE with per-half output stores.
"""

import math
import sys
from contextlib import ExitStack

sys.path.insert(0, "/opt/trn_rl_repo")

import numpy as np
import ml_dtypes

import concourse.bass as bass
from concourse import bacc
import concourse.mybir as mybir
import concourse.tile as tile

F32 = mybir.dt.float32
BF16 = mybir.dt.bfloat16
AF = mybir.ActivationFunctionType
ALU = mybir.AluOpType

B, S, E = 4, 2048, 1024
H, HD = 16, 64
HLOC = 8          # heads per core
ILOC = HLOC * HD  # 512 internal dims per core
KT = E // 128     # 8 embed k-tiles
ST = S // 128     # 16 seq tiles
NCORES = 8
SCALE = 1.0 / math.sqrt(HD)  # 1/8


def build_nc():
    nc = bacc.Bacc()

    qT_d = nc.declare_dram_parameter("qT", [E, S], BF16, isOutput=False).ap()
    kT_d = nc.declare_dram_parameter("kT", [E, S], BF16, isOutput=False).ap()
    vT_d = nc.declare_dram_parameter("vT", [E, S], BF16, isOutput=False).ap()
    wq_d = nc.declare_dram_parameter("wq", [E, ILOC], BF16, isOutput=False).ap()
    wk_d = nc.declare_dram_parameter("wk", [E, ILOC], BF16, isOutput=False).ap()
    wv_d = nc.declare_dram_parameter("wv", [E, ILOC], BF16, isOutput=False).ap()
    wo_d = nc.declare_dram_parameter("wo", [ILOC, E], BF16, isOutput=False).ap()
    bq_d = nc.declare_dram_parameter("bq", [128, 4], F32, isOutput=False).ap()
    bk_d = nc.declare_dram_parameter("bk", [128, 4], F32, isOutput=False).ap()
    out_d = nc.declare_dram_parameter("out", [S, E], BF16, isOutput=True).ap()

    with tile.TileContext(nc) as tc, ExitStack() as ctx:
        # ---- pools (PSUM: pp 2x1 + sc 2x2 + av 2x1 = 8 banks) ----
        psum = ctx.enter_context(tc.tile_pool(name="psum", bufs=2, space="PSUM"))
        av_pool = ctx.enter_context(tc.tile_pool(name="avp", bufs=2, space="PSUM"))
        qhT_pool = ctx.enter_context(tc.tile_pool(name="qhT", bufs=3))
        khT_pool = ctx.enter_context(tc.tile_pool(name="khT", bufs=3))
        vh_pool = ctx.enter_context(tc.tile_pool(name="vh", bufs=ST))
        bias_pool = ctx.enter_context(tc.tile_pool(name="bias", bufs=1))
        wpool = ctx.enter_context(tc.tile_pool(name="w_in", bufs=4))
        stage_pool = ctx.enter_context(tc.tile_pool(name="stage", bufs=8))
        stq0_pool = ctx.enter_context(tc.tile_pool(name="stq0", bufs=8))
        stqr_pool = ctx.enter_context(tc.tile_pool(name="stqr", bufs=8))
        vst_pool = ctx.enter_context(tc.tile_pool(name="vstage", bufs=3))
        exp_pool = ctx.enter_context(tc.tile_pool(name="exp", bufs=6))
        attnT_pool = ctx.enter_context(tc.tile_pool(name="attnT", bufs=4))
        small_pool = ctx.enter_context(tc.tile_pool(name="small", bufs=2))
        rr_pool = ctx.enter_context(tc.tile_pool(name="rrow", bufs=2))
        tmp_pool = ctx.enter_context(tc.tile_pool(name="tmpp", bufs=1))
        out_pool = ctx.enter_context(tc.tile_pool(name="outbuf", bufs=2))

        qhT = [qhT_pool.tile([128, S], BF16, tag="qhT", name=f"qhT{i}")
               for i in range(4)]
        khT = [khT_pool.tile([128, S], BF16, tag="khT", name=f"khT{i}")
               for i in range(4)]
        vh = [vh_pool.tile([128, HLOC * 65], BF16, tag="vh", name=f"vh{i}")
              for i in range(ST)]

        bq_t = bias_pool.tile([128, 4], F32, tag="bq")
        bk_t = bias_pool.tile([128, 4], F32, tag="bk")
        ones_t = bias_pool.tile([1, 64], BF16, tag="ones")
        nc.sync.dma_start(bq_t[:], bq_d[:])
        nc.sync.dma_start(bk_t[:], bk_d[:])
        nc.vector.memset(ones_t[:], 1.0)

        # p-state warm-up: DMA-independent dummy matmuls run while the
        # input staging DMAs are still in flight, so the TensorE clock is
        # already ramped when the first real chain fires
        warm_t = bias_pool.tile([128, 512], BF16, tag="warm")
        nc.vector.memset(warm_t[:], 0.0)
        warm_ps = psum.tile([128, 512], F32, tag="pp", name="warm")
        for _ in range(40):
            nc.tensor.matmul(warm_ps[:], lhsT=warm_t[:, 0:128],
                             rhs=warm_t[:], start=True, stop=True)

        wq_t = wpool.tile([128, KT, ILOC], BF16, tag="w")
        wk_t = wpool.tile([128, KT, ILOC], BF16, tag="w")
        wv_t = wpool.tile([128, KT, ILOC], BF16, tag="w")
        wo_t = wpool.tile([128, 4, E], BF16, tag="w")
        attnT = [attnT_pool.tile([128, S], BF16, tag="attnT",
                                 name=f"attnT{i}") for i in range(4)]

        # ---- staging: v in small tiles interleaved with k, then q ----
        # (q quarter-0 is staged separately so attention can start before
        # the rest of q lands)
        # wv + vst0/vst1 first: the very first PE work (vh chains) needs
        # only these, so the first matmul fires as early as possible
        nc.sync.dma_start(wv_t[:], wv_d.rearrange("(k p) n -> p k n", p=128))
        vst, stg_k, stg_q0, stg_qr = [], [], [], []

        def vst_dma(j):
            t = vst_pool.tile([128, KT, 256], BF16, tag="vst", name=f"vst{j}")
            nc.sync.dma_start(
                t[:],
                vT_d[:, j * 256:(j + 1) * 256].rearrange(
                    "(k p) n -> p k n", p=128))
            vst.append(t)

        vst_dma(0)
        vst_dma(1)
        nc.sync.dma_start(wk_t[:], wk_d.rearrange("(k p) n -> p k n", p=128))
        # v tiles 2-5 interleave with k; 6-7 (consumed inside qc0) follow
        # q quarter-0 so kT lands earlier for the khT m0 chains
        for j in range(8):
            if 2 <= j < 6:
                vst_dma(j)
            tk = stage_pool.tile([128, S], BF16, tag="stage")
            nc.sync.dma_start(tk[:], kT_d[j * 128:(j + 1) * 128, :])
            stg_k.append(tk)
        nc.sync.dma_start(wq_t[:], wq_d.rearrange("(k p) n -> p k n", p=128))
        for kk in range(KT):
            t = stq0_pool.tile([128, 512], BF16, tag="stq0")
            nc.sync.dma_start(t[:], qT_d[kk * 128:(kk + 1) * 128, 0:512])
            stg_q0.append(t)
        vst_dma(6)
        vst_dma(7)
        # quarter-major staging: the q1 projection fillers (running in
        # pair 0's steps) unblock after 1MB instead of 3MB
        for quarter in (1, 2, 3):
            tiles = []
            for kk in range(KT):
                t = stqr_pool.tile([128, 512], BF16, tag=f"stqr{quarter}")
                nc.sync.dma_start(
                    t[:],
                    qT_d[kk * 128:(kk + 1) * 128,
                         quarter * 512:(quarter + 1) * 512])
                tiles.append(t)
            stg_qr.append(tiles)
        nc.sync.dma_start(wo_t[:], wo_d.rearrange("(i p) n -> p i n", p=128))

        def vh_chain(st):
            """Project vh[st] from the small v-stage tiles."""
            j, off = divmod(st, 2)
            ps = psum.tile([128, 512], F32, tag="pp", name="psv")
            for kk in range(KT):
                nc.tensor.matmul(
                    ps[:],
                    lhsT=vst[j][:, kk, off * 128:off * 128 + 128],
                    rhs=wv_t[:, kk, :],
                    start=(kk == 0), stop=(kk == KT - 1),
                )
            pin = ps[:].rearrange("p (a b x) -> p a b x", b=2, x=64)
            pout = vh[st].rearrange("p (a c) -> p a c", c=130)
            nc.vector.tensor_copy(pout[:, :, 0:64], pin[:, :, 0, :])
            nc.vector.tensor_copy(pout[:, :, 65:129], pin[:, :, 1, :])
            ones = vh[st].rearrange("p (h x) -> p h x", x=65)[:, :, 64:65]
            nc.vector.memset(ones, 1.0)

        def q_src(kk, quarter):
            """Source tile + column slice for qT quarter (split staging)."""
            if quarter == 0:
                return stg_q0[kk], slice(0, 512)
            return stg_qr[quarter - 1][kk], slice(0, 512)

        def k_src(kk, quarter):
            return stg_k[kk], slice(quarter * 512, (quarter + 1) * 512)

        def proj_ops(m, which="kq", quarters=range(4), evac_act=False):
            """Closure list projecting khT/qhT m-tile chains."""
            ops = []
            srcs = {"k": ((k_src, wk_t, khT, bk_t),),
                    "q": ((q_src, wq_t, qhT, bq_t),)}
            chosen = srcs["k"] + srcs["q"] if which == "kq" else srcs[which]
            for src_f, w_t, dst, b_t in chosen:
                for quarter in quarters:
                    cols = slice(quarter * 512, (quarter + 1) * 512)
                    holder = {}

                    for kk in range(KT):
                        def mm(kk=kk, src_f=src_f, w_t=w_t, quarter=quarter,
                               holder=holder, first=(kk == 0)):
                            if first:
                                holder["ps"] = psum.tile(
                                    [128, 512], F32, tag="pp", name="psqk")
                            stg_t, scols = src_f(kk, quarter)
                            nc.tensor.matmul(
                                holder["ps"][:],
                                lhsT=w_t[:, kk, m * 128:(m + 1) * 128],
                                rhs=stg_t[:, scols],
                                start=(kk == 0), stop=(kk == KT - 1),
                            )
                        ops.append(mm)

                    def evac(dst=dst, cols=cols, b_t=b_t, m=m, holder=holder,
                             evac_act=evac_act):
                        if evac_act:
                            nc.scalar.activation(
                                dst[m][:, cols], holder["ps"][:], AF.Identity,
                                bias=b_t[:, m:m + 1], scale=1.0)
                        else:
                            nc.vector.tensor_scalar(
                                dst[m][:, cols], holder["ps"][:],
                                b_t[:, m:m + 1], None, ALU.add)
                    ops.append(evac)
            return ops

        def outproj_qt(qt, evac_act=False, big_psum=False):
            """Closure list projecting output for one 128-query tile.

            evac_act: tail mode -- c0 evac on DVE, c1 on ScalarE (parallel
            engines) and each half's output DMA fires immediately.
            big_psum: back the po accumulators with an sc-tag tile (the
            score tiles are dead in the tail), deepening the rotation so
            chains never wait on evacs.
            """
            ops = []
            holder = {}
            for c in range(2):
                for it in range(4):
                    def mm(qt=qt, c=c, it=it, holder=holder,
                           first=(c == 0 and it == 0), big_psum=big_psum):
                        if first:
                            holder["ot"] = out_pool.tile(
                                [128, 1024], BF16, tag="ot", name="ot")
                        if it == 0:
                            if big_psum:
                                if c == 0:
                                    holder["po2"] = psum.tile(
                                        [128, 1024], F32, tag="sc",
                                        name="po2")
                                holder["po"] = holder["po2"][
                                    :, c * 512:(c + 1) * 512]
                            else:
                                holder["po"] = psum.tile(
                                    [128, 512], F32, tag="pp", name="po")[:]
                        nc.tensor.matmul(
                            holder["po"],
                            lhsT=attnT[it][:, qt * 128:(qt + 1) * 128],
                            rhs=wo_t[:, it, c * 512:(c + 1) * 512],
                            start=(it == 0), stop=(it == 3),
                        )
                    ops.append(mm)

                def evac(qt=qt, c=c, holder=holder, last=(c == 1),
                         evac_act=evac_act):
                    dst = holder["ot"][:, c * 512:(c + 1) * 512]
                    if evac_act:
                        if c == 0:
                            nc.vector.tensor_copy(dst, holder["po"])
                        else:
                            nc.scalar.copy(dst, holder["po"])
                        nc.sync.dma_start(
                            out_d[qt * 128:(qt + 1) * 128,
                                  c * 512:(c + 1) * 512], dst)
                    else:
                        nc.vector.tensor_copy(dst, holder["po"])
                        if last:
                            nc.sync.dma_start(
                                out_d[qt * 128:(qt + 1) * 128, :],
                                holder["ot"][:])
                ops.append(evac)
            return ops

        def outproj_ops(qc):
            return [op for qt in range(qc * 4, qc * 4 + 4)
                    for op in outproj_qt(qt)]

        def divide_slice(g, avsA, avsB, qc, lo, hi, use_pe=False):
            """Normalize AV slice [lo:hi) (within-qc cols) into attnT[g].

            The denominator reciprocal row is replicated across the 64 hd
            partitions on the idle GPSIMD engine; the latency-critical
            tail slices keep the K=1 TensorE broadcast instead.
            """
            n = hi - lo
            qlo = qc * 512 + lo
            for avs, dsthalf in ((avsA, 0), (avsB, 1)):
                rcp = rr_pool.tile([64, 512], F32, tag="rcp", name="rcp")
                if use_pe:
                    rb = rr_pool.tile([1, 512], BF16, tag="rb", name="rb")
                    nc.vector.tensor_copy(rb[:, 0:n], avs[64:65, lo:hi])
                    bc = psum.tile([64, n], F32, tag="pp", name="bc")
                    nc.tensor.matmul(bc[:], lhsT=ones_t[0:1, :],
                                     rhs=rb[:, 0:n], start=True, stop=True)
                    nc.vector.reciprocal_approx_fast(rcp[:, 0:n], bc[:])
                else:
                    # reciprocal_approx_fast needs matching base
                    # partitions, so copy the denominator row down first
                    rbf = rr_pool.tile([1, 512], F32, tag="rrf", name="rbf")
                    nc.vector.tensor_copy(rbf[:, 0:n], avs[64:65, lo:hi])
                    rr = rr_pool.tile([1, 512], F32, tag="rrg", name="rr")
                    nc.vector.reciprocal_approx_fast(rr[:, 0:n],
                                                     rbf[:, 0:n])
                    nc.gpsimd.partition_broadcast(rcp[:, 0:n], rr[:, 0:n],
                                                  channels=64)
                if dsthalf == 0:
                    nc.vector.tensor_mul(attnT[g][0:64, qlo:qlo + n],
                                         avs[0:64, lo:hi], rcp[:, 0:n])
                else:
                    tmp = tmp_pool.tile([64, 512], BF16, tag="tmp",
                                        name="tmp")
                    nc.vector.tensor_mul(tmp[:, 0:n], avs[0:64, lo:hi],
                                         rcp[:, 0:n])
                    nc.sync.dma_start(attnT[g][64:128, qlo:qlo + n],
                                      tmp[:, 0:n])

        # ---- upfront TensorE window: vh + khT m0 + qhT q0 ----
        # (vh 12-15 ride inside qc0's first steps; their AV uses are 8+
        # steps later)
        for st in range(12):
            vh_chain(st)
        for op in proj_ops(0, "k", evac_act=True):
            op()
        for op in proj_ops(0, "q", quarters=(0,), evac_act=True):
            op()

        pending = []   # deferred division ops from the previous query chunk
        pending_tail = []  # out-proj held for the tail flush
        for g in range(4):              # head pair (2g, 2g+1)
            hA, hB = 2 * g, 2 * g + 1
            if g == 0:
                fillers = proj_ops(0, "q", quarters=(1, 2, 3)) + proj_ops(1)
            elif g == 1:
                fillers = proj_ops(2)
            elif g == 2:
                fillers = proj_ops(3, "k") + proj_ops(3, "q",
                                                      quarters=(0,))
            else:
                # pair 3's last q-proj quarters fill qc0's exp-paced
                # slack -- no out-proj is runnable before g3 finishes a qc
                fillers = proj_ops(3, "q", quarters=(1, 2, 3))
            steps_left = 4 * ST
            for qc in range(4):         # 512-query chunks
                qcols = slice(qc * 512, (qc + 1) * 512)
                avA = av_pool.tile([65, 512], F32, tag="av", name="avA")
                avB = av_pool.tile([65, 512], F32, tag="av", name="avB")
                for kt in range(ST):
                    sc = psum.tile([128, 1024], F32, tag="sc", name="sc")
                    nc.tensor.matmul(
                        sc[:, 0:512],
                        lhsT=khT[g][0:64, kt * 128:(kt + 1) * 128],
                        rhs=qhT[g][0:64, qcols],
                        start=True, stop=True,
                    )
                    nc.tensor.matmul(
                        sc[:, 512:1024],
                        lhsT=khT[g][64:128, kt * 128:(kt + 1) * 128],
                        rhs=qhT[g][64:128, qcols],
                        start=True, stop=True,
                    )
                    ex = exp_pool.tile([128, 1024], BF16, tag="exp", name="ex")
                    nc.scalar.activation(ex[:], sc[:], AF.Exp)
                    if g == 0 and qc == 0 and kt < 4:
                        vh_chain(12 + kt)
                    first, last = (kt == 0), (kt == ST - 1)
                    nc.tensor.matmul(
                        avA[0:65, :],
                        lhsT=vh[kt][:, hA * 65:hA * 65 + 65],
                        rhs=ex[:, 0:512],
                        start=first, stop=last,
                    )
                    nc.tensor.matmul(
                        avB[0:65, :],
                        lhsT=vh[kt][:, hB * 65:hB * 65 + 65],
                        rhs=ex[:, 512:1024],
                        start=first, stop=last,
                    )
                    if kt == 1:
                        # previous chunk's deferred division; out-proj
                        # fillers may only be appended after it is emitted
                        for op in pending:
                            op()
                        pending = []
                        if g == 3 and qc in (1, 2):
                            fillers.extend(outproj_ops(qc - 1))
                        if g == 3 and qc == 3:
                            # qc2's out-proj runs in the tail flush, not
                            # in qc3's steps, so the final exps stream
                            # unstarved
                            pending_tail.extend(outproj_qt(8))
                            pending_tail.extend(outproj_qt(9))
                    # pace interleaved filler work (proj / out-proj)
                    steps_left -= 1
                    n_take = -(-len(fillers) // max(steps_left, 1)) \
                        if fillers else 0
                    for _ in range(min(n_take, len(fillers))):
                        fillers.pop(0)()
                # ---- AV evacuation now; the division defers to the next
                # chunk's steps so its matmul never stalls the score stream
                avsA = small_pool.tile([65, 512], F32, tag="avs", name="avsA")
                avsB = small_pool.tile([65, 512], F32, tag="avs", name="avsB")
                nc.vector.tensor_copy(avsA[:], avA[:])
                nc.vector.tensor_copy(avsB[:], avB[:])
                if g == 3 and qc == 3:
                    # tail: qc2's remaining out-proj bridges the PE over
                    # the divisions' DVE latency; chains alternate between
                    # sc-backed and pp-backed PSUM so they never stall on
                    # evacs, and evacs split across DVE/ScalarE with
                    # per-half output stores
                    for op in fillers:
                        op()
                    fillers = []
                    for op in pending_tail:
                        op()
                    pending_tail = []
                    divide_slice(g, avsA, avsB, qc, 0, 128, use_pe=True)
                    for qt in (10, 11):
                        for op in outproj_qt(qt, evac_act=True,
                                             big_psum=(qt == 10)):
                            op()
                    for sl in range(1, 4):
                        divide_slice(g, avsA, avsB, qc,
                                     sl * 128, (sl + 1) * 128, use_pe=True)
                    for sl in range(4):
                        for op in outproj_qt(qc * 4 + sl, evac_act=True,
                                             big_psum=(sl % 2 == 0)):
                            op()
                else:
                    pending = [
                        lambda g=g, avsA=avsA, avsB=avsB, qc=qc:
                        divide_slice(g, avsA, avsB, qc, 0, 512)]
            # flush any leftover fillers for this pair
            for op in fillers:
                op()

    nc.finalize()
    return nc


def make_in_maps(q, k, v, Wq, bq, Wk, bk, Wv, bv, Wo, bo):
    """Per-core input dicts + the folded host-side bias."""
    bf = ml_dtypes.bfloat16
    qT = [np.ascontiguousarray(q[b].T).astype(bf) for b in range(B)]
    kT = [np.ascontiguousarray(k[b].T).astype(bf) for b in range(B)]
    vT = [np.ascontiguousarray(v[b].T).astype(bf) for b in range(B)]
    in_maps = []
    for c in range(NCORES):
        b, hh = divmod(c, 2)
        isl = slice(hh * ILOC, (hh + 1) * ILOC)
        bq_loc = np.ascontiguousarray(
            (bq[isl] * SCALE).reshape(4, 128).T)
        bk_loc = np.ascontiguousarray(bk[isl].reshape(4, 128).T)
        in_maps.append({
            "qT": qT[b], "kT": kT[b], "vT": vT[b],
            "wq": np.ascontiguousarray(Wq[:, isl] * SCALE).astype(bf),
            "wk": np.ascontiguousarray(Wk[:, isl]).astype(bf),
            "wv": np.ascontiguousarray(Wv[:, isl]).astype(bf),
            "wo": np.ascontiguousarray(Wo[isl, :]).astype(bf),
            "bq": bq_loc, "bk": bk_loc,
        })
    bo_eff = (bo + bv @ Wo).astype(np.float32)
    return in_maps, bo_eff


_NC_CACHE = None


def kernel(q, k, v, Wq, bq, Wk, bk, Wv, bv, Wo, bo):
    global _NC_CACHE
    from concourse.bass_utils import run_bass_kernel_spmd

    if _NC_CACHE is None:
        _NC_CACHE = build_nc()
    nc = _NC_CACHE
    in_maps, bo_eff = make_in_maps(q, k, v, Wq, bq, Wk, bk, Wv, bv, Wo, bo)
    res = run_bass_kernel_spmd(nc, in_maps, list(range(NCORES)))
    out = np.empty((B, S, E), np.float32)
    for b in range(B):
        out[b] = (res.results[2 * b]["out"].astype(np.float32)
                  + res.results[2 * b + 1]["out"].astype(np.float32)
                  + bo_eff)
    return out



# revision 47
# speedup vs baseline: 1.0083x; 1.0083x over previous
"""Multi-head attention (B=4, S=2048, E=1024, H=16, hd=64) on 8 TRN2 cores.

Sharding: core c -> batch b = c//2, head-half hh = c%2 (8 heads = 512 internal
dims).  Data parallel on B, tensor parallel on heads.  Each core computes a
partial out-projection for its batch; the host sums the two head-half partials
per batch and adds the (folded) output bias.

Device dataflow (bf16 matmuls, fp32 PSUM accumulation):
  - host pre-transposes q/k/v to (E, S) and casts to bf16 so the projection
    matmuls need no on-chip transpose.  v is staged in eight small
    [128, 8kk, 256seq] tiles that arrive progressively, so the vh projection
    overlaps the k/q staging DMA; khT m-tile 0 and qhT quarter 0 are packed
    into the same TensorE window so attention starts as soon as qT lands.
  - attention per head-PAIR (2g, 2g+1) per 512-query chunk: two K=64
    scoresT matmuls for both heads into one PSUM tile, one Exp over both
    (scale 1/8 pre-folded into Wq/bq host-side), then M=65 AV matmuls
    whose ones-column accumulates the softmax denominator in row 64.
  - division: AV promptly evacuated PSUM->SBUF (frees the accumulator);
    the denominator row's DVE fast-reciprocal is replicated across the 64
    hd partitions by the otherwise-idle GPSIMD engine (partition_broadcast)
    for the deferred steady-state divides; the latency-critical tail slices
    keep a K=1 TensorE broadcast (ones[1,64]^T @ row).  The multiply
    produces attn_outT -- exactly the lhsT needed for the out-projection
    po (q x E) = attn_outT^T @ Wo_loc.
  - ScalarE runs only the exps plus tail evacuations; steady-state
    projection evacuations ride on DVE (tensor_scalar bias-add) since the
    exp stream is the pacing engine.
  - engines run their streams in order, so projection m-tile g+1 matmuls
    are explicitly interleaved into attention pair g's steps to keep
    TensorE busy while ScalarE paces the exps.  Pair 3 has no runnable
    out-proj during its qc0 (out-proj for a query chunk needs all four
    pairs done), so qhT[3] quarters 1-3 are deferred into that slack;
    out-proj(qc) then fills g3's qc+1, with half of out-proj(qc2) riding
    in qc3's steps.
  - a p-state warm-up of 40 DMA-independent dummy matmuls runs during the
    staging window, so the first real chain executes at full TensorE clock.
  - tail (g3 qc3): divide slice 0, out-proj qt10/11 bridging the division
    latency, remaining divides, then the four final out-proj chains.
    Tail chains alternate between sc-tag (score tiles are dead by then)
    and pp-tag PSUM so they never stall on evacuations; final evacs split
    c0->DVE / c1->ScalarE with per-half output stores.
"""

import math
import sys
from contextlib import ExitStack

sys.path.insert(0, "/opt/trn_rl_repo")

import numpy as np
import ml_dtypes

import concourse.bass as bass
from concourse import bacc
import concourse.mybir as mybir
import concourse.tile as tile

F32 = mybir.dt.float32
BF16 = mybir.dt.bfloat16
AF = mybir.ActivationFunctionType
ALU = mybir.AluOpType

B, S, E = 4, 2048, 1024
H, HD = 16, 64
HLOC = 8          # heads per core
ILOC = HLOC * HD  # 512 internal dims per core
KT = E // 128     # 8 embed k-tiles
ST = S // 128     # 16 seq tiles
NCORES = 8
SCALE = 1.0 / math.sqrt(HD)  # 1/8


def build_nc():
    nc = bacc.Bacc()

    qT_d = nc.declare_dram_parameter("qT", [E, S], BF16, isOutput=False).ap()
    kT_d = nc.declare_dram_parameter("kT", [E, S], BF16, isOutput=False).ap()
    vT_d = nc.declare_dram_parameter("vT", [E, S], BF16, isOutput=False).ap()
    wq_d = nc.declare_dram_parameter("wq", [E, ILOC], BF16, isOutput=False).ap()
    wk_d = nc.declare_dram_parameter("wk", [E, ILOC], BF16, isOutput=False).ap()
    wv_d = nc.declare_dram_parameter("wv", [E, ILOC], BF16, isOutput=False).ap()
    wo_d = nc.declare_dram_parameter("wo", [ILOC, E], BF16, isOutput=False).ap()
    bq_d = nc.declare_dram_parameter("bq", [128, 4], F32, isOutput=False).ap()
    bk_d = nc.declare_dram_parameter("bk", [128, 4], F32, isOutput=False).ap()
    out_d = nc.declare_dram_parameter("out", [S, E], BF16, isOutput=True).ap()

    with tile.TileContext(nc) as tc, ExitStack() as ctx:
        # ---- pools (PSUM: pp 2x1 + sc 2x2 + av 2x1 = 8 banks) ----
        psum = ctx.enter_context(tc.tile_pool(name="psum", bufs=2, space="PSUM"))
        av_pool = ctx.enter_context(tc.tile_pool(name="avp", bufs=2, space="PSUM"))
        qhT_pool = ctx.enter_context(tc.tile_pool(name="qhT", bufs=3))
        khT_pool = ctx.enter_context(tc.tile_pool(name="khT", bufs=3))
        vh_pool = ctx.enter_context(tc.tile_pool(name="vh", bufs=ST))
        bias_pool = ctx.enter_context(tc.tile_pool(name="bias", bufs=1))
        wpool = ctx.enter_context(tc.tile_pool(name="w_in", bufs=4))
        stage_pool = ctx.enter_context(tc.tile_pool(name="stage", bufs=8))
        stq0_pool = ctx.enter_context(tc.tile_pool(name="stq0", bufs=8))
        stqr_pool = ctx.enter_context(tc.tile_pool(name="stqr", bufs=8))
        vst_pool = ctx.enter_context(tc.tile_pool(name="vstage", bufs=3))
        exp_pool = ctx.enter_context(tc.tile_pool(name="exp", bufs=6))
        attnT_pool = ctx.enter_context(tc.tile_pool(name="attnT", bufs=4))
        small_pool = ctx.enter_context(tc.tile_pool(name="small", bufs=2))
        rr_pool = ctx.enter_context(tc.tile_pool(name="rrow", bufs=2))
        tmp_pool = ctx.enter_context(tc.tile_pool(name="tmpp", bufs=1))
        out_pool = ctx.enter_context(tc.tile_pool(name="outbuf", bufs=2))

        qhT = [qhT_pool.tile([128, S], BF16, tag="qhT", name=f"qhT{i}")
               for i in range(4)]
        khT = [khT_pool.tile([128, S], BF16, tag="khT", name=f"khT{i}")
               for i in range(4)]
        vh = [vh_pool.tile([128, HLOC * 65], BF16, tag="vh", name=f"vh{i}")
              for i in range(ST)]

        bq_t = bias_pool.tile([128, 4], F32, tag="bq")
        bk_t = bias_pool.tile([128, 4], F32, tag="bk")
        ones_t = bias_pool.tile([1, 64], BF16, tag="ones")
        nc.sync.dma_start(bq_t[:], bq_d[:])
        nc.sync.dma_start(bk_t[:], bk_d[:])
        nc.vector.memset(ones_t[:], 1.0)

        # p-state warm-up: DMA-independent dummy matmuls run while the
        # input staging DMAs are still in flight, so the TensorE clock is
        # already ramped when the first real chain fires
        warm_t = bias_pool.tile([128, 512], BF16, tag="warm")
        nc.vector.memset(warm_t[:], 0.0)
        warm_ps = psum.tile([128, 512], F32, tag="pp", name="warm")
        for _ in range(40):
            nc.tensor.matmul(warm_ps[:], lhsT=warm_t[:, 0:128],
                             rhs=warm_t[:], start=True, stop=True)

        wq_t = wpool.tile([128, KT, ILOC], BF16, tag="w")
        wk_t = wpool.tile([128, KT, ILOC], BF16, tag="w")
        wv_t = wpool.tile([128, KT, ILOC], BF16, tag="w")
        wo_t = wpool.tile([128, 4, E], BF16, tag="w")
        attnT = [attnT_pool.tile([128, S], BF16, tag="attnT",
                                 name=f"attnT{i}") for i in range(4)]

        # ---- staging: v in small tiles interleaved with k, then q ----
        # (q quarter-0 is staged separately so attention can start before
        # the rest of q lands)
        # wv + vst0/vst1 first: the very first PE work (vh chains) needs
        # only these, so the first matmul fires as early as possible
        nc.sync.dma_start(wv_t[:], wv_d.rearrange("(k p) n -> p k n", p=128))
        vst, stg_k, stg_q0, stg_qr = [], [], [], []

        def vst_dma(j):
            t = vst_pool.tile([128, KT, 256], BF16, tag="vst", name=f"vst{j}")
            nc.sync.dma_start(
                t[:],
                vT_d[:, j * 256:(j + 1) * 256].rearrange(
                    "(k p) n -> p k n", p=128))
            vst.append(t)

        vst_dma(0)
        vst_dma(1)
        nc.sync.dma_start(wk_t[:], wk_d.rearrange("(k p) n -> p k n", p=128))
        # v tiles 2-5 interleave with k; 6-7 (consumed inside qc0) follow
        # q quarter-0 so kT lands earlier for the khT m0 chains
        for j in range(8):
            if 2 <= j < 6:
                vst_dma(j)
            tk = stage_pool.tile([128, S], BF16, tag="stage")
            nc.sync.dma_start(tk[:], kT_d[j * 128:(j + 1) * 128, :])
            stg_k.append(tk)
        nc.sync.dma_start(wq_t[:], wq_d.rearrange("(k p) n -> p k n", p=128))
        for kk in range(KT):
            t = stq0_pool.tile([128, 512], BF16, tag="stq0")
            nc.sync.dma_start(t[:], qT_d[kk * 128:(kk + 1) * 128, 0:512])
            stg_q0.append(t)
        vst_dma(6)
        vst_dma(7)
        # quarter-major staging: the q1 projection fillers (running in
        # pair 0's steps) unblock after 1MB instead of 3MB
        for quarter in (1, 2, 3):
            tiles = []
            for kk in range(KT):
                t = stqr_pool.tile([128, 512], BF16, tag=f"stqr{quarter}")
                nc.sync.dma_start(
                    t[:],
                    qT_d[kk * 128:(kk + 1) * 128,
                         quarter * 512:(quarter + 1) * 512])
                tiles.append(t)
            stg_qr.append(tiles)
        nc.sync.dma_start(wo_t[:], wo_d.rearrange("(i p) n -> p i n", p=128))

        def vh_chain(st):
            """Project vh[st] from the small v-stage tiles."""
            j, off = divmod(st, 2)
            ps = psum.tile([128, 512], F32, tag="pp", name="psv")
            for kk in range(KT):
                nc.tensor.matmul(
                    ps[:],
                    lhsT=vst[j][:, kk, off * 128:off * 128 + 128],
                    rhs=wv_t[:, kk, :],
                    start=(kk == 0), stop=(kk == KT - 1),
                )
            pin = ps[:].rearrange("p (a b x) -> p a b x", b=2, x=64)
            pout = vh[st].rearrange("p (a c) -> p a c", c=130)
            nc.vector.tensor_copy(pout[:, :, 0:64], pin[:, :, 0, :])
            nc.vector.tensor_copy(pout[:, :, 65:129], pin[:, :, 1, :])
            ones = vh[st].rearrange("p (h x) -> p h x", x=65)[:, :, 64:65]
            nc.vector.memset(ones, 1.0)

        def q_src(kk, quarter):
            """Source tile + column slice for qT quarter (split staging)."""
            if quarter == 0:
                return stg_q0[kk], slice(0, 512)
            return stg_qr[quarter - 1][kk], slice(0, 512)

        def k_src(kk, quarter):
            return stg_k[kk], slice(quarter * 512, (quarter + 1) * 512)

        def proj_ops(m, which="kq", quarters=range(4), evac_act=False):
            """Closure list projecting khT/qhT m-tile chains."""
            ops = []
            srcs = {"k": ((k_src, wk_t, khT, bk_t),),
                    "q": ((q_src, wq_t, qhT, bq_t),)}
            chosen = srcs["k"] + srcs["q"] if which == "kq" else srcs[which]
            for src_f, w_t, dst, b_t in chosen:
                for quarter in quarters:
                    cols = slice(quarter * 512, (quarter + 1) * 512)
                    holder = {}

                    for kk in range(KT):
                        def mm(kk=kk, src_f=src_f, w_t=w_t, quarter=quarter,
                               holder=holder, first=(kk == 0)):
                            if first:
                                holder["ps"] = psum.tile(
                                    [128, 512], F32, tag="pp", name="psqk")
                            stg_t, scols = src_f(kk, quarter)
                            nc.tensor.matmul(
                                holder["ps"][:],
                                lhsT=w_t[:, kk, m * 128:(m + 1) * 128],
                                rhs=stg_t[:, scols],
                                start=(kk == 0), stop=(kk == KT - 1),
                            )
                        ops.append(mm)

                    def evac(dst=dst, cols=cols, b_t=b_t, m=m, holder=holder,
                             evac_act=evac_act):
                        if evac_act:
                            nc.scalar.activation(
                                dst[m][:, cols], holder["ps"][:], AF.Identity,
                                bias=b_t[:, m:m + 1], scale=1.0)
                        else:
                            nc.vector.tensor_scalar(
                                dst[m][:, cols], holder["ps"][:],
                                b_t[:, m:m + 1], None, ALU.add)
                    ops.append(evac)
            return ops

        def outproj_qt(qt, evac_act=False, big_psum=False):
            """Closure list projecting output for one 128-query tile.

            evac_act: tail mode -- c0 evac on DVE, c1 on ScalarE (parallel
            engines) and each half's output DMA fires immediately.
            big_psum: back the po accumulators with an sc-tag tile (the
            score tiles are dead in the tail), deepening the rotation so
            chains never wait on evacs.
            """
            ops = []
            holder = {}
            for c in range(2):
                for it in range(4):
                    def mm(qt=qt, c=c, it=it, holder=holder,
                           first=(c == 0 and it == 0), big_psum=big_psum):
                        if first:
                            holder["ot"] = out_pool.tile(
                                [128, 1024], BF16, tag="ot", name="ot")
                        if it == 0:
                            if big_psum:
                                if c == 0:
                                    holder["po2"] = psum.tile(
                                        [128, 1024], F32, tag="sc",
                                        name="po2")
                                holder["po"] = holder["po2"][
                                    :, c * 512:(c + 1) * 512]
                            else:
                                holder["po"] = psum.tile(
                                    [128, 512], F32, tag="pp", name="po")[:]
                        nc.tensor.matmul(
                            holder["po"],
                            lhsT=attnT[it][:, qt * 128:(qt + 1) * 128],
                            rhs=wo_t[:, it, c * 512:(c + 1) * 512],
                            start=(it == 0), stop=(it == 3),
                        )
                    ops.append(mm)

                def evac(qt=qt, c=c, holder=holder, last=(c == 1),
                         evac_act=evac_act):
                    dst = holder["ot"][:, c * 512:(c + 1) * 512]
                    if evac_act:
                        if c == 0:
                            nc.vector.tensor_copy(dst, holder["po"])
                        else:
                            nc.scalar.copy(dst, holder["po"])
                        nc.sync.dma_start(
                            out_d[qt * 128:(qt + 1) * 128,
                                  c * 512:(c + 1) * 512], dst)
                    else:
                        nc.vector.tensor_copy(dst, holder["po"])
                        if last:
                            nc.sync.dma_start(
                                out_d[qt * 128:(qt + 1) * 128, :],
                                holder["ot"][:])
                ops.append(evac)
            return ops

        def outproj_ops(qc):
            return [op for qt in range(qc * 4, qc * 4 + 4)
                    for op in outproj_qt(qt)]

        def divide_slice(g, avsA, avsB, qc, lo, hi, use_pe=False):
            """Normalize AV slice [lo:hi) (within-qc cols) into attnT[g].

            The denominator reciprocal row is replicated across the 64 hd
            partitions on the idle GPSIMD engine; the latency-critical
            tail slices keep the K=1 TensorE broadcast instead.
            """
            n = hi - lo
            qlo = qc * 512 + lo
            for avs, dsthalf in ((avsA, 0), (avsB, 1)):
                rcp = rr_pool.tile([64, 512], F32, tag="rcp", name="rcp")
                if use_pe:
                    rb = rr_pool.tile([1, 512], BF16, tag="rb", name="rb")
                    nc.vector.tensor_copy(rb[:, 0:n], avs[64:65, lo:hi])
                    bc = psum.tile([64, n], F32, tag="pp", name="bc")
                    nc.tensor.matmul(bc[:], lhsT=ones_t[0:1, :],
                                     rhs=rb[:, 0:n], start=True, stop=True)
                    nc.vector.reciprocal_approx_fast(rcp[:, 0:n], bc[:])
                else:
                    # reciprocal_approx_fast needs matching base
                    # partitions, so copy the denominator row down first
                    rbf = rr_pool.tile([1, 512], F32, tag="rrf", name="rbf")
                    nc.vector.tensor_copy(rbf[:, 0:n], avs[64:65, lo:hi])
                    rr = rr_pool.tile([1, 512], F32, tag="rrg", name="rr")
                    nc.vector.reciprocal_approx_fast(rr[:, 0:n],
                                                     rbf[:, 0:n])
                    nc.gpsimd.partition_broadcast(rcp[:, 0:n], rr[:, 0:n],
                                                  channels=64)
                if dsthalf == 0:
                    nc.vector.tensor_mul(attnT[g][0:64, qlo:qlo + n],
                                         avs[0:64, lo:hi], rcp[:, 0:n])
                else:
                    tmp = tmp_pool.tile([64, 512], BF16, tag="tmp",
                                        name="tmp")
                    nc.vector.tensor_mul(tmp[:, 0:n], avs[0:64, lo:hi],
                                         rcp[:, 0:n])
                    nc.sync.dma_start(attnT[g][64:128, qlo:qlo + n],
                                      tmp[:, 0:n])

        # ---- upfront TensorE window: vh + khT m0 + qhT q0 ----
        # (vh 12-15 ride inside qc0's first steps; their AV uses are 8+
        # steps later)
        for st in range(12):
            vh_chain(st)
        for op in proj_ops(0, "k", evac_act=True):
            op()
        for op in proj_ops(0, "q", quarters=(0,), evac_act=True):
            op()

        pending = []   # deferred division ops from the previous query chunk
        for g in range(4):              # head pair (2g, 2g+1)
            hA, hB = 2 * g, 2 * g + 1
            if g == 0:
                fillers = proj_ops(0, "q", quarters=(1, 2, 3)) + proj_ops(1)
            elif g == 1:
                fillers = proj_ops(2)
            elif g == 2:
                fillers = proj_ops(3, "k") + proj_ops(3, "q",
                                                      quarters=(0,))
            else:
                # pair 3's last q-proj quarters fill qc0's exp-paced
                # slack -- no out-proj is runnable before g3 finishes a qc
                fillers = proj_ops(3, "q", quarters=(1, 2, 3))
            steps_left = 4 * ST
            for qc in range(4):         # 512-query chunks
                qcols = slice(qc * 512, (qc + 1) * 512)
                avA = av_pool.tile([65, 512], F32, tag="av", name="avA")
                avB = av_pool.tile([65, 512], F32, tag="av", name="avB")
                for kt in range(ST):
                    sc = psum.tile([128, 1024], F32, tag="sc", name="sc")
                    nc.tensor.matmul(
                        sc[:, 0:512],
                        lhsT=khT[g][0:64, kt * 128:(kt + 1) * 128],
                        rhs=qhT[g][0:64, qcols],
                        start=True, stop=True,
                    )
                    nc.tensor.matmul(
                        sc[:, 512:1024],
                        lhsT=khT[g][64:128, kt * 128:(kt + 1) * 128],
                        rhs=qhT[g][64:128, qcols],
                        start=True, stop=True,
                    )
                    ex = exp_pool.tile([128, 1024], BF16, tag="exp", name="ex")
                    nc.scalar.activation(ex[:], sc[:], AF.Exp)
                    if g == 0 and qc == 0 and kt < 4:
                        vh_chain(12 + kt)
                    first, last = (kt == 0), (kt == ST - 1)
                    nc.tensor.matmul(
                        avA[0:65, :],
                        lhsT=vh[kt][:, hA * 65:hA * 65 + 65],
                        rhs=ex[:, 0:512],
                        start=first, stop=last,
                    )
                    nc.tensor.matmul(
                        avB[0:65, :],
                        lhsT=vh[kt][:, hB * 65:hB * 65 + 65],
                        rhs=ex[:, 512:1024],
                        start=first, stop=last,
                    )
                    if kt == 1:
                        # previous chunk's deferred division; out-proj
                        # fillers may only be appended after it is emitted
                        for op in pending:
                            op()
                        pending = []
                        if g == 3 and qc in (1, 2):
                            fillers.extend(outproj_ops(qc - 1))
                        if g == 3 and qc == 3:
                            # half of qc2's out-proj rides in qc3's
                            # exp-paced slack; the rest stays in the tail
                            fillers.extend(outproj_qt(8))
                            fillers.extend(outproj_qt(9))
                    # pace interleaved filler work (proj / out-proj)
                    steps_left -= 1
                    n_take = -(-len(fillers) // max(steps_left, 1)) \
                        if fillers else 0
                    for _ in range(min(n_take, len(fillers))):
                        fillers.pop(0)()
                # ---- AV evacuation now; the division defers to the next
                # chunk's steps so its matmul never stalls the score stream
                avsA = small_pool.tile([65, 512], F32, tag="avs", name="avsA")
                avsB = small_pool.tile([65, 512], F32, tag="avs", name="avsB")
                nc.vector.tensor_copy(avsA[:], avA[:])
                nc.vector.tensor_copy(avsB[:], avB[:])
                if g == 3 and qc == 3:
                    # tail: qc2's remaining out-proj bridges the PE over
                    # the divisions' DVE latency; chains alternate between
                    # sc-backed and pp-backed PSUM so they never stall on
                    # evacs, and evacs split across DVE/ScalarE with
                    # per-half output stores
                    for op in fillers:
                        op()
                    fillers = []
                    divide_slice(g, avsA, avsB, qc, 0, 128, use_pe=True)
                    for qt in (10, 11):
                        for op in outproj_qt(qt, evac_act=True,
                                             big_psum=(qt == 10)):
                            op()
                    for sl in range(1, 4):
                        divide_slice(g, avsA, avsB, qc,
                                     sl * 128, (sl + 1) * 128, use_pe=True)
                    for sl in range(4):
                        for op in outproj_qt(qc * 4 + sl, evac_act=True,
                                             big_psum=(sl % 2 == 0)):
                            op()
                else:
                    pending = [
                        lambda g=g, avsA=avsA, avsB=avsB, qc=qc:
                        divide_slice(g, avsA, avsB, qc, 0, 512)]
            # flush any leftover fillers for this pair
            for op in fillers:
                op()

    nc.finalize()
    return nc


def make_in_maps(q, k, v, Wq, bq, Wk, bk, Wv, bv, Wo, bo):
    """Per-core input dicts + the folded host-side bias."""
    bf = ml_dtypes.bfloat16
    qT = [np.ascontiguousarray(q[b].T).astype(bf) for b in range(B)]
    kT = [np.ascontiguousarray(k[b].T).astype(bf) for b in range(B)]
    vT = [np.ascontiguousarray(v[b].T).astype(bf) for b in range(B)]
    in_maps = []
    for c in range(NCORES):
        b, hh = divmod(c, 2)
        isl = slice(hh * ILOC, (hh + 1) * ILOC)
        bq_loc = np.ascontiguousarray(
            (bq[isl] * SCALE).reshape(4, 128).T)
        bk_loc = np.ascontiguousarray(bk[isl].reshape(4, 128).T)
        in_maps.append({
            "qT": qT[b], "kT": kT[b], "vT": vT[b],
            "wq": np.ascontiguousarray(Wq[:, isl] * SCALE).astype(bf),
            "wk": np.ascontiguousarray(Wk[:, isl]).astype(bf),
            "wv": np.ascontiguousarray(Wv[:, isl]).astype(bf),
            "wo": np.ascontiguousarray(Wo[isl, :]).astype(bf),
            "bq": bq_loc, "bk": bk_loc,
        })
    bo_eff = (bo + bv @ Wo).astype(np.float32)
    return in_maps, bo_eff


_NC_CACHE = None


def kernel(q, k, v, Wq, bq, Wk, bk, Wv, bv, Wo, bo):
    global _NC_CACHE
    from concourse.bass_utils import run_bass_kernel_spmd

    if _NC_CACHE is None:
        _NC_CACHE = build_nc()
    nc = _NC_CACHE
    in_maps, bo_eff = make_in_maps(q, k, v, Wq, bq, Wk, bk, Wv, bv, Wo, bo)
    res = run_bass_kernel_spmd(nc, in_maps, list(range(NCORES)))
    out = np.empty((B, S, E), np.float32)
    for b in range(B):
        out[b] = (res.results[2 * b]["out"].astype(np.float32)
                  + res.results[2 * b + 1]["out"].astype(np.float32)
                  + bo_eff)
    return out

